# revision 1
# baseline (speedup 1.0000x reference)
"""Mixtral-style sparse MoE block on 8 Trainium2 NeuronCores.

Strategy: expert-parallel, all-bf16. The router (tiny: T x H @ H x E) runs on
the host as part of input sharding; each core is assigned one expert and
receives the tokens routed to it (padded up to a 64-token grid), transposed
to feature-major layout. The host applies the renormalized top-2 combine
weights and scatter-adds the per-expert outputs back into the full [T, H]
output.

Per-core kernel math (C = token capacity, padded):
    h  = silu(x @ w1) * (x @ w3)    # [C, F]
    y  = h @ w2                     # [C, H]
computed in feature-on-partition layout: activations are [feature, token]
so all three weight matrices are used in their natural [K, M] layout as
matmul stationary operands and the SwiGLU intermediate h lands directly in
the [F-partition, token-free] layout that the down-projection consumes.

All matmuls are bf16 (1 cycle/row at 2.4 GHz — same PE rate as fp32r but
half the bytes). That lets w1 (7.3 MB) and w2 (7.3 MB) stay RESIDENT in
SBUF for the whole kernel; only w3 is re-streamed per 512-token slice
(22 us of DMA against 143 us of PE work, consumed at a leisurely 1.5 us /
3.4 us cadence). Per-slice DMA drops from ~48 MB (old fp32r streaming
baseline, which saturated DMA at ~305 GB/s and stalled the PE) to ~9.5 MB,
making the kernel purely tensor-engine-bound. PSUM accumulates in fp32, so
accuracy stays ~3e-3 relative — far inside the 2e-2 gate.
"""

import numpy as np

H = 1024        # hidden dim
F = 3584        # FFN dim
E = 8           # experts == cores
NT = 512        # full token slice (psum bank = 512 fp32)
KH = H // 128   # 8 k-tiles over hidden
MF = F // 128   # 28 m-tiles over ffn
MH = H // 128   # 8 m-tiles over hidden (down-proj output)

_compile_cache = {}
_last_result = None  # BassKernelResults of the most recent run (for profiling)


def _slice_plan(max_cnt):
    """Token-slice widths covering max_cnt: full 512s + one exact tail.

    The tail width is the exact remainder rounded up to 4 tokens (keeps DMA
    lines 8-byte aligned); bf16 matmuls run 1 cycle/row at any free-dim
    width and LDWEIGHTS stays hidden down to ~80 ns/matmul, so a narrow
    tail costs exactly its width."""
    n_full, rem = divmod(max(max_cnt, 64), NT)
    tail = -(-rem // 4) * 4
    return (NT,) * n_full + ((tail,) if tail else ())


def _build(slices):
    """Build + compile the per-core Bass program for the given slice widths."""
    import concourse.bass as bass
    import concourse.mybir as mybir
    import concourse.tile as tile
    from concourse import bacc

    C = sum(slices)
    f32 = mybir.dt.float32
    bf16 = mybir.dt.bfloat16
    ts = bass.ts

    nc = bacc.Bacc("TRN2", target_bir_lowering=False, debug=False, num_devices=E)

    xT = nc.dram_tensor("xT", [H, C], bf16, kind="ExternalInput").ap()
    w1s = nc.dram_tensor("w1s", [MF, 128, H], bf16, kind="ExternalInput").ap()
    w3s = nc.dram_tensor("w3s", [MF, 128, H], bf16, kind="ExternalInput").ap()
    w2s = nc.dram_tensor("w2s", [MH, 128, F], bf16, kind="ExternalInput").ap()
    yT = nc.dram_tensor("yT", [H, C], bf16, kind="ExternalOutput").ap()

    xT_r = xT.rearrange("(k p) t -> p k t", p=128)
    yT_r = yT.rearrange("(m p) t -> m p t", p=128)

    with tile.TileContext(nc, trace_sim=False) as tc:
        with (
            tc.tile_pool(name="xp", bufs=2) as xp,
            tc.tile_pool(name="w1p", bufs=MF) as w1p,      # resident
            tc.tile_pool(name="w3p", bufs=8) as w3p,       # streamed per slice
            tc.tile_pool(name="w2p", bufs=MH) as w2p,      # resident
            tc.tile_pool(name="hp", bufs=MF + 2) as hp,
            tc.tile_pool(name="hsp", bufs=3) as hsp,
            tc.tile_pool(name="yp", bufs=3) as yp,
            tc.tile_pool(name="ps1p", bufs=3, space="PSUM") as ps1p,
            tc.tile_pool(name="ps3p", bufs=3, space="PSUM") as ps3p,
            tc.tile_pool(name="psyp", bufs=2, space="PSUM") as psyp,
        ):
            w1_res = [None] * MF
            w2_res = None

            # No PE warmup burst: measured per-core x-DMA arrival jitters
            # 13-19 us, so a fixed-length burst leaves a >3.4 us idle gap
            # on late cores and the HAM clock gate re-throttles — worse
            # for the max core than just starting cold once.

            offs = []
            off = 0
            for width in slices:
                offs.append((off, width))
                off += width
            for si, (off, width) in enumerate(offs):
                if si == 0:
                    # w1[0] before x: mm1(0)'s LDWEIGHTS needs it first and
                    # it is 4x smaller than the x slice
                    w1t = w1p.tile([128, H], bf16)
                    nc.sync.dma_start(w1t[:], w1s[0])
                    w1_res[0] = w1t
                xt = xp.tile([128, KH, NT], bf16)
                # one 3D-AP descriptor for the whole [H, width] x slice:
                # 8 separate per-k descriptors cost ~600 ns each of serial
                # sync-engine issue on slice 0's critical path
                w3t0 = None
                if si == 0:
                    # queue order w1[0], x(k0-1), w3[0], x(k2-7): the first
                    # mm1 chain starts ~5 us sooner on the small x chunk,
                    # and w3[0] jumps ahead of the x bulk so mm3(0) does
                    # not stall on weights behind the 1 MB transfer
                    nc.sync.dma_start(
                        xt[:, :2, :width], xT_r[:, :2, off : off + width]
                    )
                    w3t0 = w3p.tile([128, H], bf16)
                    nc.sync.dma_start(w3t0[:], w3s[0])
                    nc.sync.dma_start(
                        xt[:, 2:, :width], xT_r[:, 2:, off : off + width]
                    )
                else:
                    nc.sync.dma_start(
                        xt[:, :, :width], xT_r[:, :, off : off + width]
                    )

                h_tiles = []
                for m in range(MF):
                    if si == 0 and m > 0:
                        # lazily interleave the resident-w1 loads with the
                        # w3 stream so slice 0's mm1(m) never waits behind
                        # the whole weight prologue in the DMA queue
                        w1t = w1p.tile([128, H], bf16)
                        nc.sync.dma_start(w1t[:], w1s[m])
                        w1_res[m] = w1t
                    if si == 0 and m == 0:
                        w3t = w3t0
                    else:
                        w3t = w3p.tile([128, H], bf16)
                        nc.sync.dma_start(w3t[:], w3s[m])

                    ps1 = ps1p.tile([128, NT], f32)
                    for k in range(KH):
                        nc.tensor.matmul(
                            ps1[:, :width],
                            w1_res[m][:, ts(k, 128)],
                            xt[:, k, :width],
                            start=(k == 0),
                            stop=(k == KH - 1),
                        )
                    ps3 = ps3p.tile([128, NT], f32)
                    for k in range(KH):
                        nc.tensor.matmul(
                            ps3[:, :width],
                            w3t[:, ts(k, 128)],
                            xt[:, k, :width],
                            start=(k == 0),
                            stop=(k == KH - 1),
                        )
                    hs = hsp.tile([128, NT], f32)
                    nc.scalar.activation(
                        hs[:, :width], ps1[:, :width],
                        mybir.ActivationFunctionType.Silu,
                    )
                    ht = hp.tile([128, NT], bf16)
                    nc.vector.tensor_mul(ht[:, :width], hs[:, :width], ps3[:, :width])
                    h_tiles.append(ht)

                if si == 0:
                    # resident w2 loads: queued after slice 0's w1/w3 DMAs,
                    # needed only when the first down-proj chain starts
                    w2_res = []
                    for mh in range(MH):
                        w2t = w2p.tile([128, F], bf16)
                        nc.sync.dma_start(w2t[:], w2s[mh])
                        w2_res.append(w2t)

                for mh in range(MH):
                    psy = psyp.tile([128, NT], f32)
                    for kf in range(MF):
                        nc.tensor.matmul(
                            psy[:, :width],
                            w2_res[mh][:, ts(kf, 128)],
                            h_tiles[kf][:, :width],
                            start=(kf == 0),
                            stop=(kf == MF - 1),
                        )
                    yt = yp.tile([128, NT], bf16)
                    nc.vector.tensor_copy(yt[:, :width], psy[:, :width])
                    nc.sync.dma_start(yT_r[mh, :, off : off + width], yt[:, :width])

    nc.compile()
    return nc


def _route(x, gate_w, gate_b):
    """Host router: top-2 expert ids + renormalized combine weights."""
    logits = x.astype(np.float32) @ gate_w.astype(np.float32).T + gate_b.astype(
        np.float32
    )
    # top-2 by prob == top-2 by logit (softmax is monotonic); stable sort
    # matches jax.lax.top_k's lower-index-first tie-breaking.
    top2 = np.argsort(-logits, axis=-1, kind="stable")[:, :2]
    l2 = np.take_along_axis(logits, top2, axis=1)
    e2 = np.exp(l2 - l2.max(axis=1, keepdims=True))
    wts = e2 / e2.sum(axis=1, keepdims=True)
    return top2, wts.astype(np.float32)


def kernel(x, gate_w, gate_b, w1, w3, w2):
    import ml_dtypes
    from concourse.bass_utils import run_bass_kernel_spmd

    bf16 = ml_dtypes.bfloat16
    x = np.asarray(x, dtype=np.float32)
    T = x.shape[0]
    top2, wts = _route(x, np.asarray(gate_w), np.asarray(gate_b))

    idx_list, scale_list = [], []
    for e in range(E):
        sel = top2 == e                      # [T, 2] bool
        tok = np.nonzero(sel.any(axis=1))[0]
        idx_list.append(tok)
        # each token picks an expert at most once, so this take is unique
        which = sel[tok, 1].astype(np.int64)  # 0 if slot0, 1 if slot1
        scale_list.append(wts[tok, which])

    max_cnt = max(len(i) for i in idx_list)
    slices = _slice_plan(max_cnt)
    C = sum(slices)

    nc = _compile_cache.get(slices)
    if nc is None:
        nc = _build(slices)
        _compile_cache[slices] = nc

    w1 = np.asarray(w1, dtype=np.float32)
    w3 = np.asarray(w3, dtype=np.float32)
    w2 = np.asarray(w2, dtype=np.float32)
    x_bf = x.astype(bf16)

    in_maps = []
    for e in range(E):
        tok = idx_list[e]
        xTe = np.zeros((H, C), bf16)
        xTe[:, : len(tok)] = x_bf[tok].T
        # W[k*128+p, m*128+c] -> [m, p, k*128+c]: 2KB-contiguous lhsT tiles
        w1s_e = np.ascontiguousarray(
            w1[e].reshape(KH, 128, MF, 128).transpose(2, 1, 0, 3).reshape(MF, 128, H)
        ).astype(bf16)
        w3s_e = np.ascontiguousarray(
            w3[e].reshape(KH, 128, MF, 128).transpose(2, 1, 0, 3).reshape(MF, 128, H)
        ).astype(bf16)
        w2s_e = np.ascontiguousarray(
            w2[e].reshape(MF, 128, MH, 128).transpose(2, 1, 0, 3).reshape(MH, 128, F)
        ).astype(bf16)
        in_maps.append({"xT": xTe, "w1s": w1s_e, "w3s": w3s_e, "w2s": w2s_e})

    global _last_result
    res = run_bass_kernel_spmd(nc, in_maps, core_ids=list(range(E)))
    _last_result = res

    out = np.zeros((T, H), np.float32)
    for e in range(E):
        tok = idx_list[e]
        if len(tok) == 0:
            continue
        yTe = res.results[e]["yT"]
        out[tok] += yTe[:, : len(tok)].T.astype(np.float32) * scale_list[e][:, None]
    return out

